# revision 8
# baseline (speedup 1.0000x reference)
"""Trainium2 Bass kernel for nn_Decoder_80874234184280 (embedding_lookup).

Math (reference):
  eos  = eos_emb @ We^T + be                    [B,C,D]
  bin  = emb_table[1:1+N] @ Wb^T                [N,D]      (N = 23*8192)
  out  = relu(bin @ W1^T  +  eos @ W2^T + bfc)  [B,C,BINS,S]
with Wfc = [W1 | W2].

Algebraic folding (exact, associativity only):
  pb[n,s]   = emb_table[1+n] . M1[s]   where M1 = W1 @ Wb      [S,BOT]
  pe[b,c,s] = eos_emb[b,c] . M2[s] + bias3[s]
              where M2 = W2 @ We, bias3 = W2 @ be + bfc        [S,E],[S]
  out[b,c,n,s] = relu(pb[c*BINS+n, s] + pe[b,c,s])

So the 50-GFLOP problem collapses to two skinny matmuls plus a broadcast
add + relu over the 145MB output. Sharding: split the BINS axis (8192)
into 8 chunks of 1024 — each core reads 1/8 of the table and writes 1/8
of the output; eos (6MB) is replicated.

On-device per core (SPMD, no collectives):
  pe:  psum[3,(c,b)] = sum_e M2T[e,3].T @ eosT[e,(c,b)]  (8 K-chunks)
       pe_sb = Identity(psum + bias3[s] per-partition)
  per chromosome c, per half h (512 bins):
    pbT  psum[3,512]  = M1T[128,3].T @ Tk[128, 512-chunk]
    pbRow sbuf[1,1536] <- DMA-interleave pbT (n-major -> (n*3+s))
    outp psum[64,1536] = mask-matmul(pe_c) + ones-matmul(pbRow)  (K=3 + K=1)
    osb  sbuf[64,3072] = relu(outp)   (ScalarE / VectorE alternating)
    DMA out -> HBM [64, c, h*512:(h+1)*512, 3]
"""

import numpy as np

B, C, E, D, BOT, BINS, S = 64, 23, 1024, 1024, 128, 8192, 3
NUM_BINS = C * BINS
NCORES = 8
NB = BINS // NCORES          # 1024 bins per chromosome per core
CB = C * B                   # 1472 (c-major, b-minor) columns for pe
NSH = C * NB                 # 23552 table rows per core

_cache = {}


def _build_nc(n_chrom=C):
    import concourse.bass as bass
    import concourse.tile as tile
    from concourse import bacc, mybir

    f32 = mybir.dt.float32
    nc = bacc.Bacc("TRN2", target_bir_lowering=False, debug=False)

    tk_d = nc.dram_tensor("tk", [128, NSH], f32, kind="ExternalInput")
    eosT_d = nc.dram_tensor("eosT", [8, 128, CB], f32, kind="ExternalInput")
    m1t_d = nc.dram_tensor("m1t", [128, S], f32, kind="ExternalInput")
    m2t_d = nc.dram_tensor("m2t", [8, 128, S], f32, kind="ExternalInput")
    mask_d = nc.dram_tensor("mask3", [S, NB * S], f32, kind="ExternalInput")
    bias_d = nc.dram_tensor("bias3", [S, 1], f32, kind="ExternalInput")
    ones_d = nc.dram_tensor("ones64", [1, B], f32, kind="ExternalInput")
    out_d = nc.dram_tensor("out", [B, C, NB, S], f32, kind="ExternalOutput")

    with tile.TileContext(nc) as tc:
        with (
            tc.tile_pool(name="const", bufs=1) as constp,
            tc.tile_pool(name="eos", bufs=1) as eosp,
            tc.tile_pool(name="pe", bufs=1) as pep,
            tc.tile_pool(name="tkp", bufs=3) as tkp,
            tc.tile_pool(name="pbrow", bufs=3) as pbrowp,
            tc.tile_pool(name="osb", bufs=3) as osbp,
            tc.tile_pool(name="pbps", bufs=2, space="PSUM") as pbps,
            tc.tile_pool(name="outps", bufs=2, space="PSUM") as outps,
        ):
            # ---- constants ----
            m1t = constp.tile([128, S], f32)
            nc.sync.dma_start(m1t[:], m1t_d[:])
            m2t = constp.tile([128, 8, S], f32)
            nc.sync.dma_start(m2t[:], m2t_d.rearrange("k p s -> p k s"))
            mask = constp.tile([S, NB * S], f32)
            nc.sync.dma_start(mask[:], mask_d[:])
            bias3 = constp.tile([S, 1], f32)
            nc.sync.dma_start(bias3[:], bias_d[:])
            ones64 = constp.tile([1, B], f32)
            nc.sync.dma_start(ones64[:], ones_d[:])

            # ---- phase A: pe[s, (c,b)] ----
            eos_sb = eosp.tile([128, 8, CB], f32)
            nc.sync.dma_start(eos_sb[:], eosT_d.rearrange("k p n -> p k n"))
            pe_sb = pep.tile([S, CB], f32)
            for q in range(3):  # 1472 = 512+512+448
                n0, n1 = q * 512, min((q + 1) * 512, CB)
                pe_ps = pbps.tile([S, 512], f32, tag="pb")
                for ec in range(8):
                    nc.tensor.matmul(
                        pe_ps[:, : n1 - n0],
                        m2t[:, ec, :],
                        eos_sb[:, ec, n0:n1],
                        start=(ec == 0),
                        stop=(ec == 7),
                    )
                nc.scalar.activation(
                    pe_sb[:, n0:n1],
                    pe_ps[:, : n1 - n0],
                    mybir.ActivationFunctionType.Identity,
                    bias=bias3[:],
                )

            # ---- phase B: per chromosome ----
            for c in range(n_chrom):
                tk = tkp.tile([128, NB], f32)
                nc.sync.dma_start(tk[:], tk_d[:, c * NB : (c + 1) * NB])
                pbrow = pbrowp.tile([1, NB * S], f32)
                osb = osbp.tile([B, NB * S], f32)
                for h in range(2):
                    pb_ps = pbps.tile([S, 512], f32, tag="pb")
                    nc.tensor.matmul(
                        pb_ps[:], m1t[:], tk[:, h * 512 : (h + 1) * 512],
                        start=True, stop=True,
                    )
                    pbt_sb = pbrowp.tile([S, 512], f32, tag="pbt")
                    nc.scalar.copy(pbt_sb[:], pb_ps[:])
                    # interleave [3,512](s,n) -> [1,1536] j=n*3+s
                    dst = pbrow[:, h * 1536 : (h + 1) * 1536].rearrange(
                        "p (n s) -> p n s", s=S
                    )
                    for s in range(S):
                        nc.sync.dma_start(dst[:, :, s], pbt_sb[s : s + 1, :])
                    op = outps.tile([B, 1536], f32)
                    for q in range(3):
                        nc.tensor.matmul(
                            op[:, q * 512 : (q + 1) * 512],
                            pe_sb[:, c * B : (c + 1) * B],
                            mask[:, h * 1536 + q * 512 : h * 1536 + (q + 1) * 512],
                            start=True, stop=False,
                        )
                        nc.tensor.matmul(
                            op[:, q * 512 : (q + 1) * 512],
                            ones64[:],
                            pbrow[:, h * 1536 + q * 512 : h * 1536 + (q + 1) * 512],
                            start=False, stop=True,
                        )
                    dst_sb = osb[:, h * 1536 : (h + 1) * 1536]
                    if (2 * c + h) % 2 == 0:
                        nc.scalar.activation(
                            dst_sb, op[:], mybir.ActivationFunctionType.Relu
                        )
                    else:
                        nc.vector.tensor_scalar_max(dst_sb, op[:], 0.0)
                nc.sync.dma_start(
                    out_d[:, c, :, :].rearrange("b n s -> b (n s)"), osb[:]
                )
    nc.compile()
    return nc


def _prep(eos_emb, emb_table, Wb, We, be, Wfc, bfc):
    W1 = Wfc[:, :D].astype(np.float64)
    W2 = Wfc[:, D:].astype(np.float64)
    M1 = (W1 @ Wb.astype(np.float64)).astype(np.float32)          # [S,BOT]
    M2 = (W2 @ We.astype(np.float64)).astype(np.float32)          # [S,E]
    bias3 = (W2 @ be.astype(np.float64) + bfc).astype(np.float32)  # [S]

    m1t = np.ascontiguousarray(M1.T)                               # [128,S]
    m2t = np.ascontiguousarray(M2.T.reshape(8, 128, S))            # [8,128,S]
    # eosT: [E, (c,b)] c-major then split e into 8x128
    eosT = np.ascontiguousarray(
        eos_emb.transpose(2, 1, 0).reshape(E, CB).reshape(8, 128, CB)
    )
    mask3 = np.zeros((S, NB * S), dtype=np.float32)
    for s in range(S):
        mask3[s, s::S] = 1.0
    bias_c = np.ascontiguousarray(bias3.reshape(S, 1))
    ones64 = np.ones((1, B), dtype=np.float32)

    # per-core table shards, k-major: [128, 23552]
    t = emb_table[1 : 1 + NUM_BINS].reshape(C, NCORES, NB, BOT)
    tks = [
        np.ascontiguousarray(t[:, r].reshape(NSH, BOT).T) for r in range(NCORES)
    ]
    common = {
        "eosT": eosT, "m1t": m1t, "m2t": m2t, "mask3": mask3,
        "bias3": bias_c, "ones64": ones64,
    }
    return [dict(common, tk=tks[r]) for r in range(NCORES)]


def kernel(eos_emb, emb_table, Wb, We, be, Wfc, bfc):
    from concourse.bass_utils import run_bass_kernel_spmd

    if "nc" not in _cache:
        _cache["nc"] = _build_nc()
    nc = _cache["nc"]
    in_maps = _prep(eos_emb, emb_table, Wb, We, be, Wfc, bfc)
    res = run_bass_kernel_spmd(
        nc, in_maps, core_ids=list(range(NCORES)),
        trace=_cache.get("trace", False),
    )
    _cache["exec_time_ns"] = res.exec_time_ns
    _cache["res"] = res
    out = np.empty((B, C, BINS, S), dtype=np.float32)
    for r in range(NCORES):
        out[:, :, r * NB : (r + 1) * NB, :] = res.results[r]["out"]
    return out
